# revision 57
# baseline (speedup 1.0000x reference)
"""Causal multi-head attention (B=4, S=2048, H=16, Dh=64) on 8 TRN2 NeuronCores.

Sharding: core c = (batch b=c//2, head-group g=c%2). Each core handles heads
8g..8g+7 of its batch over the FULL 2048-query sequence (head/tensor
parallelism). K/V/Q projections then only cover this core's 8 heads — no
duplicated projection work across the pair (the query-parallel alternative
computes each batch's full K/V projection twice). The WO projection is
computed as a PARTIAL product (contraction over this core's 512 O-columns
only); the host sums the two partials per batch and adds WO_bias during the
gather, so no cross-core collective is needed.

Compute dtype: bf16 matmul inputs, fp32 PSUM accumulation, bf16 partial out.

Device pipeline per core (single fused Tile program):
  - DVE memsets a warmup tile immediately; the PE runs ~22 dummy matmuls on
    it from t~2.5us so the tensor engine's p-state is ramped to 2.4 GHz by
    the time the first real operands land (a cold/idle PE runs at 1.2 GHz
    for its first ~5.6us otherwise);
  - input DMAs stream from t=0 in first-need order (wv, vT head-chunk, wk,
    kT chunk 0, wq, qT chunk 0, ...);
  - V proj (token-major, interleaved ones column per head = softmax
    denominator trick): one N=512 pass per token tile (8 heads x 64 + the
    ones columns survive a memset); V bias applied with one DVE add from a
    host-replicated [128, 520] tile; KT/QT projection groups for the first
    key/query chunk are woven between the late V groups;
  - attention loop is query-chunk-major: for qc in 0..3, for hp in 0..3
    (head pair), score both heads of hp against each 128-key tile kt in one
    [128,1024] PSUM pair tile -> exp on ScalarE (no max subtraction: logits
    ~N(0,1); masked lanes are multiplied to exact 0 like the reference) ->
    fixed lower-triangular [128,128] mask multiply on diagonal key tiles ->
    AV accumulation (denominator lands in PSUM row 64 via the ones column);
    KT/QT groups for chunk qc+1 and WO groups for query tiles of qc-1 are
    popped between attention blocks as dependency-free PE filler, one right
    at each (hp,qc) boundary so the PE stays dense while DVE evacuates the
    O accumulators;
  - normalization: reciprocal chain on DVE at push time, PE-side broadcast
    (ones-matmul) + DVE multiply deferred one block so operands are ready;
  - WO partial projection per 128-query tile: 2x4 N=512 matmuls contracting
    this core's 512 O-columns, bf16 out, no bias (host adds it).
"""

import numpy as np
import ml_dtypes

B = 4
S = 2048
D = 1024
H = 16
DH = 64
NCORES = 8
NH = 8          # heads per core
NHP = 4         # head pairs per core
NKT = 16        # 128-key tiles
NQC = 4         # 512-query chunks
P = 128

_CACHE = {}


def _build():
    import concourse.mybir as mybir
    import concourse.tile as tile
    from concourse import bacc

    dt = mybir.dt
    BF = dt.bfloat16
    F32 = dt.float32
    AF = mybir.ActivationFunctionType

    nc = bacc.Bacc("TRN2", target_bir_lowering=False, debug=False,
                   num_devices=NCORES)

    qT_d = nc.declare_dram_parameter("qT", [D, S], BF, isOutput=False)
    kT_d = nc.declare_dram_parameter("kT", [D, S], BF, isOutput=False)
    vT_d = nc.declare_dram_parameter("vT", [D, S], BF, isOutput=False)
    wq_d = nc.declare_dram_parameter("wq", [D, 512], BF, isOutput=False)
    wk_d = nc.declare_dram_parameter("wk", [D, 512], BF, isOutput=False)
    wv_d = nc.declare_dram_parameter("wv", [D, 512], BF, isOutput=False)
    wo_d = nc.declare_dram_parameter("wo", [512, D], BF, isOutput=False)
    bq_d = nc.declare_dram_parameter("bq", [P, NHP], F32, isOutput=False)
    bk_d = nc.declare_dram_parameter("bk", [P, NHP], F32, isOutput=False)
    vbr_d = nc.declare_dram_parameter("vbr", [P, 65 * NH], BF, isOutput=False)
    mask_d = nc.declare_dram_parameter("masks", [P, 2 * P], BF, isOutput=False)
    out_d = nc.declare_dram_parameter("out", [S, D], BF, isOutput=True)

    with tile.TileContext(nc) as tc:
        with tc.tile_pool(name="persist", bufs=1) as pp:
            # ---- persistent SBUF tensors ----
            # V augmented: per 128-token tile t, head h at cols [65h, 65h+64)
            # plus ones at col 65h+64 (softmax denominator via matmul).
            Vaug_sb = [pp.tile([P, 65 * NH], BF, name=f"Vaug{t}",
                               tag=f"Vaug{t}") for t in range(NKT)]
            # projected K^T/Q^T per head pair: rows = 2 heads x 64 d
            KT_sb = [pp.tile([P, S], BF, name=f"KT{h}", tag=f"KT{h}")
                     for h in range(NHP)]
            QT_sb = [pp.tile([P, S], BF, name=f"QT{h}", tag=f"QT{h}")
                     for h in range(NHP)]
            OT_sb = [pp.tile([P, S], BF, name=f"OT{h}", tag=f"OT{h}")
                     for h in range(NHP)]
            # mask row 0 is all-ones (f >= 0), so it doubles as the ones
            # row-vector for K=1 broadcast matmuls
            mask_sb = pp.tile([P, 2 * P], BF, name="masksb", tag="masksb")
            ones_sb = mask_sb
            bq_sb = pp.tile([P, NHP], F32, name="bqsb", tag="bqsb")
            bk_sb = pp.tile([P, NHP], F32, name="bksb", tag="bksb")
            vb_sb = pp.tile([P, 65 * NH], BF, name="vbsb", tag="vbsb")
            warm_sb = pp.tile([P, 512], BF, name="warmsb", tag="warmsb")

            # Attention-stage pools open BEFORE the V phase so their SBUF is
            # disjoint from vstage: the kT/qT input DMAs then stream at t=0
            # instead of WAR-waiting for V-proj to release its buffers.
            astack = (
                tc.tile_pool(name="astage", bufs=1),
                tc.tile_pool(name="ppool", bufs=3),
                tc.tile_pool(name="npool", bufs=1),
            )
            asp, ppool, npool = [p.__enter__() for p in astack]
            vstack = tc.tile_pool(name="vstage", bufs=1)
            vsp = vstack.__enter__()

            wv_sb = vsp.tile([P, 8 * 512], BF, name="wvbig", tag="wvbig")
            vT_sb = vsp.tile([P, 8 * S], BF, name="vTbig", tag="vTbig")
            wk_sb = asp.tile([P, 8 * 512], BF, name="wkbig", tag="wkbig")
            kT_sb = asp.tile([P, 8 * S], BF, name="kTbig", tag="kTbig")
            wq_sb = asp.tile([P, 8 * 512], BF, name="wqbig", tag="wqbig")
            qT_sb = asp.tile([P, 8 * S], BF, name="qTbig", tag="qTbig")

            # DMA issue order == need order. Triggers serialize on the Sync
            # engine (~1us each), so the first transfers are exactly what the
            # first V groups consume.
            def _chunk(dst_big, src_d, c0, c1, n=S):
                dst = dst_big[:].rearrange("p (f c) -> p f c", c=n)[
                    :, :, c0:c1]
                sd = src_d[:, c0:c1].rearrange("(f p) c -> p f c", p=P)
                nc.sync.dma_start(dst, sd)

            def _whole(dst_big, src_d, n):
                dst = dst_big[:].rearrange("p (f c) -> p f c", c=n)
                sd = src_d[:, :].rearrange("(f p) c -> p f c", p=P)
                nc.sync.dma_start(dst, sd)

            _whole(wv_sb, wv_d, 512)
            _chunk(vT_sb, vT_d, 0, 256)
            nc.sync.dma_start(mask_sb[:], mask_d[:, :])
            nc.sync.dma_start(vb_sb[:], vbr_d[:, :])
            _chunk(vT_sb, vT_d, 256, 512)
            _chunk(vT_sb, vT_d, 512, 1024)
            _chunk(vT_sb, vT_d, 1024, 1536)
            _chunk(vT_sb, vT_d, 1536, 2048)
            _whole(wk_sb, wk_d, 512)
            _chunk(kT_sb, kT_d, 0, 512)
            nc.sync.dma_start(bq_sb[:], bq_d[:, :])
            nc.sync.dma_start(bk_sb[:], bk_d[:, :])
            _whole(wq_sb, wq_d, 512)
            _chunk(qT_sb, qT_d, 0, 512)
            _chunk(kT_sb, kT_d, 512, 1024)
            _chunk(qT_sb, qT_d, 512, 1024)
            _chunk(kT_sb, kT_d, 1024, 1536)
            _chunk(qT_sb, qT_d, 1024, 1536)
            _chunk(kT_sb, kT_d, 1536, 2048)
            _chunk(qT_sb, qT_d, 1536, 2048)

            with (
                # PSUM: "spair" [P,1024] (2 banks) x 2 + "sps" [P,512]
                # (1 bank) x 2 + "oe"/"oo" [65,512] (1 bank) x 1 each
                # = 8 banks total.
                tc.tile_pool(name="spsum", bufs=2, space="PSUM") as sps,
                tc.tile_pool(name="opsum", bufs=1, space="PSUM") as ops,
            ):
                wo_sb = [None]
                stp_box = [None]

                # ---- PE p-state warmup: dummy matmuls on a memset tile ----
                nc.vector.memset(warm_sb[:], 1.0)
                # the PE's first instruction lands at ~8us (boot barrier +
                # preamble + memset wait); 18 ramp-clock matmuls fill
                # exactly to the ~12.5us arrival of wv+vT -- more would
                # delay the V phase behind its data
                for _ in range(18):
                    wps = sps.tile([P, 512], F32, name="sps", tag="sps")
                    nc.tensor.matmul(wps[:], lhsT=warm_sb[:, 0:P],
                                     rhs=warm_sb[:], start=True, stop=True)

                def _v_group(t):
                    # only the 8 ones-columns need initialization; the V
                    # copy below overwrites everything else
                    nc.vector.memset(
                        Vaug_sb[t][:].rearrange("p (h w) -> p h w",
                                                w=65)[:, :, DH:65], 1.0)
                    ps = sps.tile([P, 512], F32, name="sps", tag="sps")
                    for fi in range(8):
                        nc.tensor.matmul(
                            ps[:],
                            lhsT=vT_sb[:, S * fi + P * t:S * fi + P * t + P],
                            rhs=wv_sb[:, 512 * fi:512 * fi + 512],
                            start=(fi == 0), stop=(fi == 7))
                    src_ap = ps[:].rearrange("p (h w) -> p h w", w=DH)
                    vaug3 = Vaug_sb[t][:].rearrange("p (h w) -> p h w", w=65)
                    nc.vector.tensor_copy(vaug3[:, :, 0:DH], src_ap)
                    # bias via a preloaded partition-replicated tile; ones
                    # cols are 0 in vb so the memset 1.0 survives. Runs on
                    # the otherwise-idle Pool engine (off the critical path:
                    # Vaug[t] is consumed much later by the AV matmuls).
                    nc.gpsimd.tensor_add(Vaug_sb[t][:], Vaug_sb[t][:],
                                         vb_sb[:])

                def _kt_group(hp, nck):
                    ps = sps.tile([P, 512], F32, name="sps", tag="sps")
                    for fi in range(8):
                        nc.tensor.matmul(
                            ps[:],
                            lhsT=wk_sb[:, 512 * fi + P * hp:
                                       512 * fi + P * hp + P],
                            rhs=kT_sb[:, S * fi + 512 * nck:
                                      S * fi + 512 * nck + 512],
                            start=(fi == 0), stop=(fi == 7))
                    # bias-add evac on ScalarE for the early chunks (ACT is
                    # idle/light there and DVE is the 90%+ engine in qc0/1;
                    # Identity shares Exp's activation table so no reload);
                    # nck3 pops in qc2 where ACT is the busier engine
                    if nck <= 2:
                        nc.scalar.activation(
                            KT_sb[hp][:, 512 * nck:512 * nck + 512],
                            ps[:], AF.Identity, bias=bk_sb[:, hp:hp + 1])
                    else:
                        nc.vector.tensor_scalar(
                            out=KT_sb[hp][:, 512 * nck:512 * nck + 512],
                            in0=ps[:], scalar1=bk_sb[:, hp:hp + 1],
                            scalar2=None, op0=mybir.AluOpType.add)

                def _qt_group(hp, nck):
                    ps = sps.tile([P, 512], F32, name="sps", tag="sps")
                    for fi in range(8):
                        nc.tensor.matmul(
                            ps[:],
                            lhsT=wq_sb[:, 512 * fi + P * hp:
                                       512 * fi + P * hp + P],
                            rhs=qT_sb[:, S * fi + 512 * nck:
                                      S * fi + 512 * nck + 512],
                            start=(fi == 0), stop=(fi == 7))
                    if nck <= 2:
                        nc.scalar.activation(
                            QT_sb[hp][:, 512 * nck:512 * nck + 512],
                            ps[:], AF.Identity, bias=bq_sb[:, hp:hp + 1])
                    else:
                        nc.vector.tensor_scalar(
                            out=QT_sb[hp][:, 512 * nck:512 * nck + 512],
                            in0=ps[:], scalar1=bq_sb[:, hp:hp + 1],
                            scalar2=None, op0=mybir.AluOpType.add)

                def _wo_group(qt, tail=False):
                    st = stp_box[0].tile([P, D], BF, name="st", tag="st",
                                         bufs=3)
                    for half in range(2):
                        ps = sps.tile([P, 512], F32, name="sps", tag="sps")
                        for f in range(NHP):
                            nc.tensor.matmul(
                                ps[:],
                                lhsT=OT_sb[f][:, P * qt:P * qt + P],
                                rhs=wo_sb[0][:, D * f + 512 * half:
                                             D * f + 512 * half + 512],
                                start=(f == 0), stop=(f == NHP - 1))
                        # tail groups split the evac across DVE/ScalarE
                        # (both idle there) to cut drain latency; a single
                        # DMA trigger per group -- triggers serialize at
                        # ~0.6us each on the Sync engine
                        hsl = slice(512 * half, 512 * half + 512)
                        if tail and half == 1:
                            nc.scalar.copy(st[:, hsl], ps[:])
                        else:
                            nc.vector.tensor_copy(st[:, hsl], ps[:])
                    nc.sync.dma_start(out_d[P * qt:P * qt + P, :], st[:])

                # pending normalizations: (hp, qc, ob_sb, rb_sb). The DVE
                # reciprocal chain runs at push time; only the broadcast
                # matmul + multiply (finish) is deferred, so by the time the
                # PE reaches the selector matmul its rb operand is long done.
                pend = []

                def _push_norm(nhp, nqc, o_e, o_o):
                    # reciprocal chain FIRST so a finish that drains soon
                    # after (tail, keep=0 points) finds rb ready; the big ob
                    # copies follow on the in-order DVE queue
                    rb = npool.tile([1, 1024], BF, name="rbsb", tag="rbsb",
                                    bufs=2)
                    d = npool.tile([1, 512], F32, name="dsb", tag="dsb",
                                   bufs=1)
                    rf = npool.tile([1, 512], F32, name="rfsb", tag="rfsb",
                                    bufs=1)
                    with nc.allow_low_precision(
                            reason="recip feeds bf16 output"):
                        for hh, o_ps in ((0, o_e), (1, o_o)):
                            nc.vector.tensor_copy(d[:],
                                                  o_ps[DH:DH + 1, 0:512])
                            nc.vector.reciprocal_approx_fast(rf[:], d[:])
                            nc.vector.tensor_copy(
                                rb[0:1, 512 * hh:512 * hh + 512], rf[:])
                    ob = npool.tile([P, 512], BF, name="osb", tag="osb",
                                    bufs=2)
                    nc.vector.tensor_copy(ob[0:DH, :], o_e[0:DH, 0:512])
                    nc.vector.tensor_copy(ob[DH:P, :], o_o[0:DH, 0:512])
                    pend.append((nhp, nqc, ob, rb))

                def _finish_norm():
                    nhp, nqc, ob, rb = pend.pop(0)
                    b_ps = sps.tile([P, 512], F32, name="sps", tag="sps")
                    nc.tensor.matmul(
                        b_ps[0:DH, 0:512],
                        lhsT=ones_sb[0:1, 0:DH],
                        rhs=rb[0:1, 0:512],
                        start=True, stop=True)
                    nc.tensor.matmul(
                        b_ps[DH:P, 0:512],
                        lhsT=ones_sb[0:1, 0:DH],
                        rhs=rb[0:1, 512:1024],
                        start=True, stop=True)
                    # the multiply reads the broadcast straight from PSUM
                    # (mixed PSUM/SBUF inputs are allowed) -- saves a
                    # [128,512] DVE copy per normalization
                    nc.vector.tensor_mul(
                        OT_sb[nhp][:, 512 * nqc:512 * nqc + 512],
                        ob[:], b_ps[:, 0:512])

                # ================== V phase (all 16 tiles) ==================
                # weave the first KT/QT chunk groups between late V groups so
                # attention qc0 starts immediately after; their DVE bias-adds
                # land early in the in-order DVE queue.
                # weaves sit late so their wk/kT/wq/qT DMAs (ordered after
                # the V-phase inputs) have landed by the time the PE arrives
                for t in range(NKT):
                    _v_group(t)
                    if t == 13:
                        _kt_group(0, 0)
                        _kt_group(1, 0)
                    elif t == 15:
                        _kt_group(2, 0)
                        _kt_group(3, 0)
                for hq in range(NHP):
                    _qt_group(hq, 0)

                # V stage fully consumed: release its SBUF and preload the
                # WO weights + output staging there
                vstack.__exit__(None, None, None)
                wostack = tc.tile_pool(name="wostage", bufs=1)
                wop = wostack.__enter__()
                wo_sb[0] = wop.tile([P, NHP * D], BF, name="wobig",
                                    tag="wobig")
                nc.sync.dma_start(
                    wo_sb[0][:].rearrange("p (f c) -> p f c", c=D),
                    wo_d[:, :].rearrange("(f p) c -> p f c", p=P))
                stp_box[0] = wop

                # ========== attention: query-chunk-major main loop ==========
                def _emit_sc(qc, hp, kt):
                    # scores + exp + causal mask for one key tile; the AV
                    # consuming pt is emitted one-plus iterations later so
                    # the exp always overlaps PE work (depth-2 pipeline,
                    # hoisted across block boundaries)
                    c0 = max(P * kt, 512 * qc)
                    c1 = 512 * qc + 512
                    w = c1 - c0
                    # both heads in one tile at 512-stride so the two
                    # K=64 score matmuls pack into disjoint halves
                    sp = sps.tile([P, 1024], F32, name="spair",
                                  tag="spair")
                    for h in range(2):
                        nc.tensor.matmul(
                            sp[:, 512 * h:512 * h + w],
                            lhsT=KT_sb[hp][DH * h:DH * h + DH,
                                           P * kt:P * kt + P],
                            rhs=QT_sb[hp][DH * h:DH * h + DH, c0:c1],
                            start=True, stop=True)
                    pt = ppool.tile([P, 1024], BF, name="pt", tag="pt")
                    if w >= 160:  # one call incl. the dead gap
                        nc.scalar.activation(pt[:, 0:512 + w],
                                             sp[:, 0:512 + w], AF.Exp)
                    else:
                        for h in range(2):
                            nc.scalar.activation(
                                pt[:, 512 * h:512 * h + w],
                                sp[:, 512 * h:512 * h + w], AF.Exp)
                    if c0 == P * kt:  # diagonal 128-col key tile
                        pm = pt[:, 0:1024].rearrange(
                            "p (h w) -> p h w", h=2)[:, :, 0:P]
                        mm = mask_sb[:].rearrange(
                            "p (h w) -> p h w", h=2)
                        nc.vector.tensor_mul(pm, pm, mm)
                    return c0, pt

                blocks = [(qc, hp) for qc in range(NQC)
                          for hp in range(NHP)]
                fillers = []
                pro = []  # scores hoisted from the previous block
                for bi, (qc, hp) in enumerate(blocks):
                    if hp == 0:
                        fillers = []
                        if qc + 1 < NQC:
                            fillers += [
                                (lambda h=h, n=qc + 1: _kt_group(h, n))
                                for h in range(NHP)]
                            nq = 2 if qc == 2 else NHP
                            fillers += [
                                (lambda h=h, n=qc + 1: _qt_group(h, n))
                                for h in range(nq)]
                        else:
                            # qt(2,3)/qt(3,3) held back from qc2 so qc3 has
                            # OT-free early fillers and wo10+wo11 survive to
                            # cover the tail's norm-finish DVE chains
                            fillers += [lambda: _qt_group(2, 3),
                                        lambda: _qt_group(3, 3)]
                        if qc >= 1:
                            # WO for query tiles of chunk qc-1. Safe: chunk
                            # qc-1's last norm is finished by this chunk's
                            # hp1 drain (hp0 drain for qc3), and the wo pops
                            # only reach the list tail after that.
                            fillers += [
                                (lambda q=q: _wo_group(q))
                                for q in range(4 * (qc - 1), 4 * qc)]

                    kts = list(range(min(NKT, 4 * (qc + 1))))
                    nkts = len(kts)
                    # drain deferred normalizations, always keeping the
                    # freshest so a finish never waits on its DVE chain.
                    # qc3's wo fillers need chunk-2 fully normalized: wo8
                    # first pops at hp1-ki0, and hp1's drain here finishes
                    # qc2-hp3 (pushed a full block earlier, chain long done)
                    # right before that -- no forced early drain needed.
                    while len(pend) > 1:
                        _finish_norm()
                    o_e = ops.tile([65, 512], F32, name="oe", tag="oe")
                    o_o = ops.tile([65, 512], F32, name="oo", tag="oo")
                    scs = pro
                    pro = []
                    if not scs:
                        scs.append(_emit_sc(qc, hp, kts[0]))
                    for ki, kt in enumerate(kts):
                        if ki + 1 < nkts:
                            if len(scs) <= ki + 1:
                                scs.append(_emit_sc(qc, hp, kts[ki + 1]))
                        elif bi + 1 < len(blocks):
                            # hoist the next block's first score+exp across
                            # the boundary: its exp runs during this block's
                            # drain/fillers, so the next block's first AV
                            # never waits on ScalarE. The KT/QT projections
                            # it reads were popped earlier this chunk (hp0/
                            # hp1), so program order is safe.
                            nqc, nhp = blocks[bi + 1]
                            pro.append(_emit_sc(nqc, nhp, 0))
                        if (ki == 0 and fillers
                                and not (qc == 3 and hp == 3)):
                            # boundary filler between the scores and the
                            # first AV: the PE chews it while ScalarE runs
                            # the exp and DVE drains the previous block's O
                            # accumulators. qc3-hp3 skips its pop so TWO wo
                            # groups survive to cover the tail's norm-finish
                            # DVE chains.
                            fillers.pop(0)()
                        c0, pt = scs[ki]
                        for h in range(2):
                            o_ps = o_e if h == 0 else o_o
                            nc.tensor.matmul(
                                o_ps[0:65, c0 - 512 * qc:512],
                                lhsT=Vaug_sb[kt][:, 65 * (2 * hp + h):
                                                 65 * (2 * hp + h) + 65],
                                rhs=pt[:, 512 * h:512 * h + 512 - c0
                                       + 512 * qc],
                                start=(ki == 0), stop=(ki == nkts - 1),
                                skip_group_check=True)
                        # interior pops are rationed so every upcoming
                        # (hp,qc) boundary still gets one filler to chew
                        # during the DVE PSUM evacuation (qc3 reserves one
                        # extra for after the last block)
                        res = (max(2, NHP - hp) if qc == 3
                               else NHP - 1 - hp)
                        if ki % 4 == 3 and len(fillers) > res:
                            fillers.pop(0)()
                    # emit the second hoisted score after this block's last
                    # AV (its PSUM slot is only free once the previous exp
                    # has been consumed)
                    if bi + 1 < len(blocks):
                        nqc, nhp = blocks[bi + 1]
                        if min(NKT, 4 * (nqc + 1)) > 1:
                            pro.append(_emit_sc(nqc, nhp, 1))
                    # evacuate O accumulators to SBUF right away (frees the
                    # PSUM banks for the next block's AVs) and kick off the
                    # reciprocal chain on DVE
                    _push_norm(hp, qc, o_e, o_o)
                    if hp == NHP - 1 and qc < NQC - 1:
                        while fillers:
                            fillers.pop(0)()

                # =============== WO projection tail (qt 12-15) ==============
                # interleave the reserved wo groups with the final norm
                # drains: each finish then runs ~2us after its push, with
                # the DVE reciprocal/evac chain long complete
                while fillers or pend:
                    if fillers:
                        fillers.pop(0)()
                    if pend:
                        _finish_norm()
                for qt in range(12, 16):
                    _wo_group(qt, tail=True)

            wostack.__exit__(None, None, None)
            for p in reversed(astack):
                p.__exit__(None, None, None)

    nc.compile()
    return nc


def _get_nc():
    if "nc" not in _CACHE:
        _CACHE["nc"] = _build()
    return _CACHE["nc"]


def _make_in_maps(q, k, v, WQ, WQ_bias, WK, WK_bias, WV, WV_bias, WO, WO_bias):
    bf = ml_dtypes.bfloat16
    scale = np.float32(1.0 / np.sqrt(DH))
    # triangular mask for diagonal key tiles: [p, f] = (f >= p); row 0 is
    # all-ones and doubles as the ones vector for broadcast matmuls
    pgrid = np.arange(P)[:, None]
    fgrid = np.arange(P)[None, :]
    m = (fgrid >= pgrid).astype(np.float32)
    masks = np.ascontiguousarray(np.concatenate([m, m], axis=1)).astype(bf)

    kT = [np.ascontiguousarray(k[b].T).astype(bf) for b in range(B)]
    vT = [np.ascontiguousarray(v[b].T).astype(bf) for b in range(B)]
    qT = [np.ascontiguousarray(q[b].T).astype(bf) for b in range(B)]

    in_maps = []
    for c in range(NCORES):
        b, g = c // 2, c % 2
        cs = slice(512 * g, 512 * g + 512)
        wq = np.ascontiguousarray(WQ[:, cs] * scale).astype(bf)
        wk = np.ascontiguousarray(WK[:, cs]).astype(bf)
        wv = np.ascontiguousarray(WV[:, cs]).astype(bf)
        wo = np.ascontiguousarray(WO[cs, :]).astype(bf)
        bq = np.ascontiguousarray(
            (WQ_bias[cs] * scale).reshape(NHP, P).T).astype(np.float32)
        bk = np.ascontiguousarray(WK_bias[cs].reshape(NHP, P).T).astype(
            np.float32)
        vrow = np.zeros(65 * NH, dtype=np.float32)
        for h in range(NH):
            vrow[65 * h:65 * h + DH] = WV_bias[DH * (NH * g + h):
                                               DH * (NH * g + h) + DH]
        vbr = np.ascontiguousarray(np.tile(vrow, (P, 1))).astype(bf)
        in_maps.append({
            "qT": qT[b], "kT": kT[b], "vT": vT[b],
            "wq": wq, "wk": wk, "wv": wv, "wo": wo,
            "bq": bq, "bk": bk, "vbr": vbr, "masks": masks,
        })
    return in_maps


def run(inputs, trace=False):
    from concourse.bass_utils import run_bass_kernel_spmd

    nc = _get_nc()
    in_maps = _make_in_maps(**inputs)
    res = run_bass_kernel_spmd(nc, in_maps, core_ids=list(range(NCORES)),
                               trace=trace)
    bias = np.asarray(inputs["WO_bias"], dtype=np.float32)
    out = np.zeros((B, S, D), dtype=np.float32)
    for b in range(B):
        out[b] = (np.asarray(res.results[2 * b]["out"]).astype(np.float32)
                  + np.asarray(res.results[2 * b + 1]["out"]).astype(
                      np.float32)
                  + bias)
    return out, res


def kernel(**inputs):
    out, _ = run(inputs, trace=False)
    return out
